# revision 5
# baseline (speedup 1.0000x reference)
"""LoRA attention kernel for Trainium2, batch-sharded across 8 NeuronCores.

Strategy:
  - Data parallel: batch B=8 -> one batch element per core.
  - LoRA factors are folded into Wqkv on the host (exact algebra, float64):
        q' = q @ (I + Aq Bq)  ==>  Wq' = (I + Aq Bq).T @ Wq   (per head)
  - All matmuls run as float32r (fp32 with 11-bit mantissa, full PE rate).
  - q,k are produced transposed ([head_dim, tokens]) directly from x^T so the
    score matmuls need no on-chip transposes. v is produced in natural layout
    with an extra all-ones column per head, so the attention-value matmul
    accumulates the softmax denominators for free in its last output row.
  - Scores are computed transposed, s[k, q]; softmax normalization is applied
    to the (small) attention output instead of the score matrix.
  - The output projection consumes the transposed attention output directly
    and produces y in natural layout; bias is fused into the PSUM drain.
"""
import numpy as np

import concourse.bass as bass
import concourse.bacc as bacc
import concourse.mybir as mybir
import concourse.tile as tile
from concourse.bass_utils import run_bass_kernel_spmd

F32 = mybir.dt.float32
F32R = mybir.dt.float32r
EXP = mybir.ActivationFunctionType.Exp

B, N, C, H, HD = 8, 1024, 768, 12, 64
CT = C // 128           # 6 contraction tiles over C
TT = N // 128           # 8 token tiles
QC = N // 512           # 2 query chunks of 512
KT = N // 128           # 8 key tiles of 128
EC = 2                  # output-projection feature chunks of 384
SCALE = HD ** -0.5
N_CORES = 8

_NC_CACHE = None


def _build():
    nc = bacc.Bacc(None, target_bir_lowering=False)

    xT = nc.dram_tensor("xT", [C, N], F32R, kind="ExternalInput")
    wqk = nc.dram_tensor("wqk", [H, CT, 128, 128], F32R, kind="ExternalInput")
    wv = nc.dram_tensor("wv", [CT, 128, C], F32R, kind="ExternalInput")
    wpt = nc.dram_tensor("wpt", [CT, 128, C], F32R, kind="ExternalInput")
    bias = nc.dram_tensor("bias", [1, C], F32, kind="ExternalInput")
    y = nc.dram_tensor("y", [N, C], F32, kind="ExternalOutput")

    with tile.TileContext(nc) as tc:
        with (
            tc.tile_pool(name="xt", bufs=CT) as xt_pool,
            tc.tile_pool(name="wqkp", bufs=2 * CT) as wqk_pool,
            tc.tile_pool(name="wvp", bufs=CT) as wv_pool,
            tc.tile_pool(name="wptp", bufs=CT) as wpt_pool,
            tc.tile_pool(name="vaug", bufs=TT) as vaug_pool,
            tc.tile_pool(name="stp", bufs=3) as st_pool,
            tc.tile_pool(name="ktp", bufs=3) as kt_pool,
            tc.tile_pool(name="expp", bufs=10) as exp_pool,
            tc.tile_pool(name="ivp", bufs=2) as iv_pool,
            tc.tile_pool(name="bcp", bufs=3) as bc_pool,
            tc.tile_pool(name="ostp", bufs=3) as ost_pool,
            tc.tile_pool(name="outp", bufs=CT) as out_pool,
            tc.tile_pool(name="yp", bufs=3) as y_pool,
            tc.tile_pool(name="cst", bufs=1) as cst_pool,
            tc.tile_pool(name="proj_ps", bufs=2, space="PSUM") as proj_ps,
            tc.tile_pool(name="sc_ps", bufs=3, space="PSUM") as sc_ps,
            tc.tile_pool(name="av_ps", bufs=2, space="PSUM") as av_ps,
        ):
            # ---- constants / global loads -------------------------------
            bias_bc = cst_pool.tile([128, C], F32, tag="biasbc")
            nc.sync.dma_start(out=bias_bc, in_=bias[:, :].to_broadcast([128, C]))
            ones12 = cst_pool.tile([128, H], F32, tag="ones12")
            nc.vector.memset(ones12, 1.0)

            xt = []
            for c in range(CT):
                t = xt_pool.tile([128, N], F32R, tag="xt")
                nc.sync.dma_start(out=t, in_=xT[c * 128:(c + 1) * 128, :])
                xt.append(t)

            wvt = []
            for c in range(CT):
                t = wv_pool.tile([128, C], F32R, tag="wv")
                nc.sync.dma_start(out=t, in_=wv[c, :, :])
                wvt.append(t)

            # ---- phase 1: v_aug[tt] = [v | 1] per head, natural layout --
            vaug = []
            for tt in range(TT):
                va = vaug_pool.tile([128, H * 65], F32R, tag="vaug")
                for half in range(2):
                    pv = proj_ps.tile([128, 384], F32, tag="mmps")
                    for c in range(CT):
                        nc.tensor.matmul(
                            pv,
                            xt[c][:, tt * 128:(tt + 1) * 128],
                            wvt[c][:, half * 384:(half + 1) * 384],
                            start=(c == 0), stop=(c == CT - 1),
                        )
                    dst = bass.AP(tensor=va.tensor,
                                  offset=va.offset + half * 6 * 65,
                                  ap=[va.ap[0], [65, 6], [1, 64]])
                    nc.vector.tensor_copy(dst, pv)
                ones_ap = bass.AP(tensor=va.tensor, offset=va.offset + 64,
                                  ap=[va.ap[0], [65, H]])
                nc.vector.tensor_copy(ones_ap, ones12)
                vaug.append(va)

            # ---- output accumulator tiles (c-major, [128, N]) -----------
            outT = [out_pool.tile([128, N], F32R, tag="outT", name=f"outT{i}")
                    for i in range(CT)]

            # ---- phase 2: per-head attention ----------------------------
            for h in range(H):
                wts = []
                for c in range(CT):
                    wt = wqk_pool.tile([128, 128], F32R, tag="wqk")
                    nc.sync.dma_start(out=wt, in_=wqk[h, c, :, :])
                    wts.append(wt)

                # q (rows 0-63) and k (rows 64-127), transposed layout
                st = st_pool.tile([128, N], F32R, tag="st")
                for qc in range(QC):
                    pqk = proj_ps.tile([128, 512], F32, tag="mmps")
                    for c in range(CT):
                        nc.tensor.matmul(
                            pqk, wts[c], xt[c][:, qc * 512:(qc + 1) * 512],
                            start=(c == 0), stop=(c == CT - 1),
                        )
                    nc.vector.tensor_copy(st[:, qc * 512:(qc + 1) * 512], pqk)

                # move k rows down to a base-0 tile (partition shift via DMA)
                kt_t = kt_pool.tile([64, N], F32R, tag="kt")
                nc.sync.dma_start(out=kt_t, in_=st[64:128, :])

                for qc in range(QC):
                    q_sl = st[0:64, qc * 512:(qc + 1) * 512]
                    av = av_ps.tile([65, 512], F32, tag="av")
                    for kt in range(KT):
                        ps_s = sc_ps.tile([128, 512], F32, tag="sc")
                        nc.tensor.matmul(
                            ps_s, kt_t[:, kt * 128:(kt + 1) * 128], q_sl,
                            start=True, stop=True,
                        )
                        et = exp_pool.tile([128, 512], F32R, tag="exp")
                        nc.scalar.activation(out=et, in_=ps_s, func=EXP,
                                             scale=SCALE)
                        nc.tensor.matmul(
                            av, vaug[kt][:, h * 65:(h + 1) * 65], et,
                            start=(kt == 0), stop=(kt == KT - 1),
                        )
                    # row 64 of av = softmax denominators for this q chunk
                    ivr = iv_pool.tile([65, 512], F32, tag="ivr")
                    nc.vector.reciprocal(ivr[64:65, :], av[64:65, :])
                    iv0 = iv_pool.tile([1, 512], F32, tag="iv0")
                    nc.sync.dma_start(out=iv0, in_=ivr[64:65, :])
                    bc = bc_pool.tile([64, 512], F32, tag="bc")
                    nc.gpsimd.partition_broadcast(bc, iv0)

                    ct_i = h // 2
                    if h % 2 == 0:
                        nc.vector.tensor_mul(
                            outT[ct_i][0:64, qc * 512:(qc + 1) * 512],
                            av[0:64, :], bc)
                    else:
                        ost = ost_pool.tile([64, 512], F32R, tag="ost")
                        nc.vector.tensor_mul(ost, av[0:64, :], bc)
                        nc.sync.dma_start(
                            out=outT[ct_i][64:128, qc * 512:(qc + 1) * 512],
                            in_=ost)

            # ---- phase 3: output projection -----------------------------
            wptt = []
            for c in range(CT):
                t = wpt_pool.tile([128, C], F32R, tag="wpt")
                nc.sync.dma_start(out=t, in_=wpt[c, :, :])
                wptt.append(t)

            for tt in range(TT):
                ysb = y_pool.tile([128, C], F32, tag="y")
                for ec in range(EC):
                    py = proj_ps.tile([128, 384], F32, tag="mmps")
                    for c in range(CT):
                        nc.tensor.matmul(
                            py,
                            outT[c][:, tt * 128:(tt + 1) * 128],
                            wptt[c][:, ec * 384:(ec + 1) * 384],
                            start=(c == 0), stop=(c == CT - 1),
                        )
                    nc.vector.tensor_add(ysb[:, ec * 384:(ec + 1) * 384], py,
                                         bias_bc[:, ec * 384:(ec + 1) * 384])
                nc.sync.dma_start(out=y[tt * 128:(tt + 1) * 128, :], in_=ysb)

    nc.finalize()
    return nc


def _get_nc():
    global _NC_CACHE
    if _NC_CACHE is None:
        _NC_CACHE = _build()
    return _NC_CACHE


def _host_prep(x, Wqkv, Wproj, bproj, Aq, Bq, Av, Bv):
    """Fold LoRA into the weights and lay everything out for the kernel."""
    W = Wqkv.astype(np.float64)
    Wq = W[0:C].reshape(H, HD, C)
    Wk = W[C:2 * C].reshape(H, HD, C)
    Wv_ = W[2 * C:3 * C].reshape(H, HD, C)
    ABq = Aq.astype(np.float64) @ Bq.astype(np.float64)   # [HD, HD]
    ABv = Av.astype(np.float64) @ Bv.astype(np.float64)
    Wq = Wq + np.einsum('ed,hec->hdc', ABq, Wq)           # (I+AB).T @ Wq per head
    Wv_ = Wv_ + np.einsum('ed,hec->hdc', ABv, Wv_)

    # wqk[h, c] = [K=c-rows(128), M = q_h cols(64) ++ k_h cols(64)]
    wqk = np.empty((H, CT, 128, 128), np.float32)
    for h in range(H):
        for c in range(CT):
            cs = slice(c * 128, (c + 1) * 128)
            wqk[h, c, :, 0:64] = Wq[h][:, cs].T.astype(np.float32)
            wqk[h, c, :, 64:128] = Wk[h][:, cs].astype(np.float32).T

    # wv[c] = [K=c-rows(128), all 768 v output features]
    WvT = Wv_.reshape(C, C).T.astype(np.float32)          # [c_in, v_out]
    wv = np.ascontiguousarray(WvT.reshape(CT, 128, C))

    # wpt[c] = Wproj.T c-tiles: [K=c(128), e(768)]
    WpT = Wproj.astype(np.float32).T                      # [c, e]
    wpt = np.ascontiguousarray(WpT.reshape(CT, 128, C))

    bias = bproj.astype(np.float32).reshape(1, C)

    per_core = []
    for b in range(B):
        xTb = np.ascontiguousarray(x[b].astype(np.float32).T)   # [C, N]
        per_core.append({"xT": xTb, "wqk": wqk, "wv": wv, "wpt": wpt,
                         "bias": bias})
    return per_core


def kernel(x, Wqkv, Wproj, bproj, Aq, Bq, Av, Bv, _trace=False):
    x = np.asarray(x)
    in_maps = _host_prep(np.asarray(x), np.asarray(Wqkv), np.asarray(Wproj),
                         np.asarray(bproj), np.asarray(Aq), np.asarray(Bq),
                         np.asarray(Av), np.asarray(Bv))
    nc = _get_nc()
    res = run_bass_kernel_spmd(nc, in_maps, core_ids=list(range(N_CORES)),
                               trace=_trace)
    out = np.stack([res.results[b]["y"] for b in range(B)], axis=0)
    if _trace:
        kernel._last_result = res
    return out.astype(np.float32)


# revision 9
# speedup vs baseline: 1.3355x; 1.3355x over previous
"""LoRA attention kernel for Trainium2, batch-sharded across 8 NeuronCores.

Strategy:
  - Data parallel: batch B=8 -> one batch element per core.
  - LoRA factors are folded into Wqkv on the host (exact algebra, float64):
        q' = q @ (I + Aq Bq)  ==>  Wq' = (I + Aq Bq).T @ Wq   (per head)
  - All matmuls run as float32r (fp32 with 11-bit mantissa, full PE rate).
  - q,k are produced transposed ([head_dim, tokens]) directly from x^T so the
    score matmuls need no on-chip transposes. v is produced in natural layout
    with an extra all-ones column per head, so the attention-value matmul
    accumulates the softmax denominators for free in its last output row.
  - Scores are computed transposed, s[k, q]; softmax normalization is applied
    to the (small) attention output instead of the score matrix.
  - The output projection consumes the transposed attention output directly
    and produces y in natural layout; bias is fused into the PSUM drain.
"""
import numpy as np

import concourse.bass as bass
import concourse.bacc as bacc
import concourse.mybir as mybir
import concourse.tile as tile
from concourse.bass_utils import run_bass_kernel_spmd

F32 = mybir.dt.float32
F32R = mybir.dt.float32r
EXP = mybir.ActivationFunctionType.Exp

B, N, C, H, HD = 8, 1024, 768, 12, 64
CT = C // 128           # 6 contraction tiles over C
TT = N // 128           # 8 token tiles
QC = N // 512           # 2 query chunks of 512
KT = N // 128           # 8 key tiles of 128
EC = 2                  # output-projection feature chunks of 384
SCALE = HD ** -0.5
N_CORES = 8

_NC_CACHE = None


def _build():
    nc = bacc.Bacc(None, target_bir_lowering=False)

    xT = nc.dram_tensor("xT", [C, N], F32R, kind="ExternalInput")
    wqk = nc.dram_tensor("wqk", [H, CT, 128, 128], F32R, kind="ExternalInput")
    wv = nc.dram_tensor("wv", [CT, 128, C], F32R, kind="ExternalInput")
    wpt = nc.dram_tensor("wpt", [CT, 128, C], F32R, kind="ExternalInput")
    bias = nc.dram_tensor("bias", [1, C], F32, kind="ExternalInput")
    y = nc.dram_tensor("y", [N, C], F32, kind="ExternalOutput")

    from contextlib import ExitStack
    with tile.TileContext(nc) as tc:
        with ExitStack() as ctx:
            pool = lambda name, bufs, **kw: ctx.enter_context(
                tc.tile_pool(name=name, bufs=bufs, **kw))
            xt_pool = pool("xt", CT)
            wqk_pool = pool("wqkp", 2 * CT)
            wv_pool = pool("wvp", CT)
            wpt_pool = pool("wptp", CT)
            vaug_pool = pool("vaug", TT)
            st_pool = pool("stp", 3)
            kt_pool = pool("ktp", 3)
            exp_pool = pool("expp", 10)
            avs_pool = pool("avsp", 4)
            iv_pool = pool("ivp", 3)
            bc_pool = pool("bcp", 3)
            ost_pool = pool("ostp", 3)
            out_pool = pool("outp", CT)
            y_pool = pool("yp", 3)
            cst_pool = pool("cst", 1)
            proj_ps = pool("proj_ps", 2, space="PSUM")
            sc_ps = pool("sc_ps", 3, space="PSUM")
            av_ps = pool("av_ps", 3, space="PSUM")
            # ---- constants / global loads -------------------------------
            xt = []
            for c in range(CT):
                t = xt_pool.tile([128, N], F32R, tag="xt", name=f"xt{c}")
                nc.sync.dma_start(out=t, in_=xT[c * 128:(c + 1) * 128, :])
                xt.append(t)

            bias_bc = cst_pool.tile([128, C], F32, tag="biasbc")
            nc.sync.dma_start(out=bias_bc, in_=bias[:, :].to_broadcast([128, C]))
            ones12 = cst_pool.tile([128, H], F32, tag="ones12")
            nc.vector.memset(ones12, 1.0)

            def load_wqk(h):
                wts = []
                for c in range(CT):
                    wt = wqk_pool.tile([128, 128], F32R, tag="wqk",
                                       name=f"wqk{h}_{c}")
                    nc.sync.dma_start(out=wt, in_=wqk[h, c, :, :])
                    wts.append(wt)
                return wts

            wts0 = load_wqk(0)

            wvt = []
            for c in range(CT):
                t = wv_pool.tile([128, C], F32R, tag="wv", name=f"wv{c}")
                nc.sync.dma_start(out=t, in_=wv[c, :, :])
                wvt.append(t)

            # ---- per-head q/k projection --------------------------------
            def qk_project(h, wts):
                """q (rows 0-63) and k (rows 64-127), transposed layout."""
                st = st_pool.tile([128, N], F32R, tag="st", name=f"st{h}")
                for qc in range(QC):
                    pqk = proj_ps.tile([128, 512], F32, tag="mmps",
                                       name=f"pqk{h}_{qc}")
                    for c in range(CT):
                        nc.tensor.matmul(
                            pqk, wts[c], xt[c][:, qc * 512:(qc + 1) * 512],
                            start=(c == 0), stop=(c == CT - 1),
                        )
                    nc.vector.tensor_copy(st[:, qc * 512:(qc + 1) * 512], pqk)
                # move k rows down to a base-0 tile (partition shift via DMA)
                kt_t = kt_pool.tile([64, N], F32R, tag="kt", name=f"kt{h}")
                nc.sync.dma_start(out=kt_t, in_=st[64:128, :])
                return st, kt_t

            st0, kt0 = qk_project(0, wts0)

            # ---- v_aug[tt] = [v | 1] per head, natural layout -----------
            vaug = []
            for tt in range(TT):
                va = vaug_pool.tile([128, H * 65], F32R, tag="vaug",
                                    name=f"vaug{tt}")
                for half in range(2):
                    pv = proj_ps.tile([128, 384], F32, tag="mmps",
                                      name=f"pv{tt}_{half}")
                    for c in range(CT):
                        nc.tensor.matmul(
                            pv,
                            xt[c][:, tt * 128:(tt + 1) * 128],
                            wvt[c][:, half * 384:(half + 1) * 384],
                            start=(c == 0), stop=(c == CT - 1),
                        )
                    dst = bass.AP(tensor=va.tensor,
                                  offset=va.offset + half * 6 * 65,
                                  ap=[va.ap[0], [65, 6], [1, 64]])
                    nc.vector.tensor_copy(dst, pv)
                ones_ap = bass.AP(tensor=va.tensor, offset=va.offset + 64,
                                  ap=[va.ap[0], [65, H]])
                nc.vector.tensor_copy(ones_ap, ones12)
                vaug.append(va)

            # ---- output accumulator tiles (c-major, [128, N]) -----------
            outT = [out_pool.tile([128, N], F32R, tag="outT", name=f"outT{i}")
                    for i in range(CT)]

            # ---- per-head attention -------------------------------------
            for h in range(H):
                if h == 0:
                    st, kt_t = st0, kt0
                else:
                    st, kt_t = qk_project(h, load_wqk(h))

                for qc in range(QC):
                    q_sl = st[0:64, qc * 512:(qc + 1) * 512]
                    av = av_ps.tile([65, 512], F32, tag="av",
                                    name=f"av{h}_{qc}")
                    for kt in range(KT):
                        ps_s = sc_ps.tile([128, 512], F32, tag="sc",
                                          name=f"sc{h}_{qc}_{kt}")
                        nc.tensor.matmul(
                            ps_s, kt_t[:, kt * 128:(kt + 1) * 128], q_sl,
                            start=True, stop=True,
                        )
                        et = exp_pool.tile([128, 512], F32R, tag="exp",
                                           name=f"exp{h}_{qc}_{kt}")
                        nc.scalar.activation(out=et, in_=ps_s, func=EXP,
                                             scale=SCALE)
                        nc.tensor.matmul(
                            av, vaug[kt][:, h * 65:(h + 1) * 65], et,
                            start=(kt == 0), stop=(kt == KT - 1),
                        )
                    # drain the whole psum quickly to release the bank; the
                    # normalization then runs off the PE critical path
                    avs = avs_pool.tile([65, 512], F32, tag="avs",
                                        name=f"avs{h}_{qc}")
                    nc.vector.tensor_copy(avs, av)
                    # row 64 of avs = softmax denominators for this q chunk.
                    # DMA-shift them to partition 0, then fast-reciprocal and
                    # broadcast (both require base partition 0).
                    sm0 = iv_pool.tile([1, 512], F32, tag="sm0",
                                       name=f"sm0{h}_{qc}")
                    nc.sync.dma_start(out=sm0, in_=avs[64:65, :])
                    iv0 = iv_pool.tile([1, 512], F32, tag="iv0",
                                       name=f"iv0{h}_{qc}")
                    nc.vector.reciprocal_approx_fast(out=iv0, in_=sm0)
                    bc = bc_pool.tile([64, 512], F32, tag="bc",
                                      name=f"bc{h}_{qc}")
                    nc.gpsimd.partition_broadcast(bc, iv0)

                    ct_i = h // 2
                    if h % 2 == 0:
                        nc.vector.tensor_mul(
                            outT[ct_i][0:64, qc * 512:(qc + 1) * 512],
                            avs[0:64, :], bc)
                    else:
                        ost = ost_pool.tile([64, 512], F32R, tag="ost",
                                            name=f"ost{h}_{qc}")
                        nc.vector.tensor_mul(ost, avs[0:64, :], bc)
                        nc.sync.dma_start(
                            out=outT[ct_i][64:128, qc * 512:(qc + 1) * 512],
                            in_=ost)

            # ---- output projection --------------------------------------
            wptt = []
            for c in range(CT):
                t = wpt_pool.tile([128, C], F32R, tag="wpt", name=f"wpt{c}")
                nc.sync.dma_start(out=t, in_=wpt[c, :, :])
                wptt.append(t)

            for tt in range(TT):
                ysb = y_pool.tile([128, C], F32, tag="y", name=f"y{tt}")
                for ec in range(EC):
                    py = proj_ps.tile([128, 384], F32, tag="mmps",
                                      name=f"py{tt}_{ec}")
                    for c in range(CT):
                        nc.tensor.matmul(
                            py,
                            outT[c][:, tt * 128:(tt + 1) * 128],
                            wptt[c][:, ec * 384:(ec + 1) * 384],
                            start=(c == 0), stop=(c == CT - 1),
                        )
                    nc.vector.tensor_add(ysb[:, ec * 384:(ec + 1) * 384], py,
                                         bias_bc[:, ec * 384:(ec + 1) * 384])
                nc.sync.dma_start(out=y[tt * 128:(tt + 1) * 128, :], in_=ysb)

    nc.finalize()
    return nc


def _get_nc():
    global _NC_CACHE
    if _NC_CACHE is None:
        _NC_CACHE = _build()
    return _NC_CACHE


def _host_prep(x, Wqkv, Wproj, bproj, Aq, Bq, Av, Bv):
    """Fold LoRA into the weights and lay everything out for the kernel."""
    W = Wqkv.astype(np.float64)
    Wq = W[0:C].reshape(H, HD, C)
    Wk = W[C:2 * C].reshape(H, HD, C)
    Wv_ = W[2 * C:3 * C].reshape(H, HD, C)
    ABq = Aq.astype(np.float64) @ Bq.astype(np.float64)   # [HD, HD]
    ABv = Av.astype(np.float64) @ Bv.astype(np.float64)
    Wq = Wq + np.einsum('ed,hec->hdc', ABq, Wq)           # (I+AB).T @ Wq per head
    Wv_ = Wv_ + np.einsum('ed,hec->hdc', ABv, Wv_)

    # wqk[h, c] = [K=c-rows(128), M = q_h cols(64) ++ k_h cols(64)]
    wqk = np.empty((H, CT, 128, 128), np.float32)
    for h in range(H):
        for c in range(CT):
            cs = slice(c * 128, (c + 1) * 128)
            wqk[h, c, :, 0:64] = Wq[h][:, cs].T.astype(np.float32)
            wqk[h, c, :, 64:128] = Wk[h][:, cs].astype(np.float32).T

    # wv[c] = [K=c-rows(128), all 768 v output features]
    WvT = Wv_.reshape(C, C).T.astype(np.float32)          # [c_in, v_out]
    wv = np.ascontiguousarray(WvT.reshape(CT, 128, C))

    # wpt[c] = Wproj.T c-tiles: [K=c(128), e(768)]
    WpT = Wproj.astype(np.float32).T                      # [c, e]
    wpt = np.ascontiguousarray(WpT.reshape(CT, 128, C))

    bias = bproj.astype(np.float32).reshape(1, C)

    per_core = []
    for b in range(B):
        xTb = np.ascontiguousarray(x[b].astype(np.float32).T)   # [C, N]
        per_core.append({"xT": xTb, "wqk": wqk, "wv": wv, "wpt": wpt,
                         "bias": bias})
    return per_core


def kernel(x, Wqkv, Wproj, bproj, Aq, Bq, Av, Bv, _trace=False):
    x = np.asarray(x)
    in_maps = _host_prep(np.asarray(x), np.asarray(Wqkv), np.asarray(Wproj),
                         np.asarray(bproj), np.asarray(Aq), np.asarray(Bq),
                         np.asarray(Av), np.asarray(Bv))
    nc = _get_nc()
    res = run_bass_kernel_spmd(nc, in_maps, core_ids=list(range(N_CORES)),
                               trace=_trace)
    out = np.stack([res.results[b]["y"] for b in range(B)], axis=0)
    if _trace:
        kernel._last_result = res
    return out.astype(np.float32)
